# revision 8
# baseline (speedup 1.0000x reference)
"""Trainium2 Bass kernel for nn_Denoiser_73598559584966.

Full-sequence self-attention (Q=K=V, no scaling) over x: [4, 16, 16, 16, 64]
  t = x.reshape(B, 4096, 64); out = softmax(t @ t^T) @ t

Sharding: 8 cores = 4 batches x 2 query-halves. Each core: 2048 queries
vs the full 4096 keys/values of its batch. No collectives.

Device algorithm per core (scores kept transposed: [keys, queries]),
all matmul operands bf16, PSUM accumulation fp32. K is pre-scaled on
host by A = 2^7/ln2 so the QK matmul directly produces z = A*s (s in
log2 units scaled to the bf16 exponent grid). Key tiles processed in
packed pairs (ktA on PE rows 0-63, ktB on rows 64-127: the two
contraction-64 matmuls run concurrently in disjoint row-halves):
  z[kt, q] = (A*K)_kt . q          bf16, contraction 64, one pass
  p = exp(s - bias)   half tiles:  ScalarE  exp(z*(1/A) - bias) -> bf16
                      other half:  VectorE  Schraudolph fast exp:
                                   i16 = int(max(z + b, 0)); p = bits(i16)
                                   read as bf16 (exponent+7-bit mantissa;
                                   ~3% rel err on p, harmless: softmax
                                   here is near-one-hot on the diagonal)
  O^T[65, q] += (V_kt|1)^T P_kt    bf16 x bf16 -> fp32; row 64 = denom
Device returns O^T [65, 2048]; the host epilogue divides rows 0..63 by
row 64 and transposes while gathering shards (O(N*C) marshaling).

bias is chosen on host from max token norm^2 (the dominant diagonal
score) so both exp paths stay in range: z + b in (0, 32640) for every
realizable score, and exp(s - bias) cannot overflow.

Loop order is key-pair-major over 2-chunk superchunks so the first
pairs only need the first khs/vpk DMA group; later groups stream in
under compute (input DMA ~2 MB/core).
"""
import numpy as np

B_, D_, H_, W_, C_ = 4, 16, 16, 16, 64
NTOK = D_ * H_ * W_          # 4096 tokens per batch
NQ = NTOK // 2               # 2048 queries per core
NCORES = 8
NKT = NTOK // 128            # 32 key tiles
NPAIR = NKT // 2             # 16 packed key-tile pairs
NCH = 4                      # query chunks per core
CHW = NQ // NCH              # 512 queries per chunk
NG = 4                       # DMA groups over key tiles
GKT = NKT // NG              # 8 key tiles per group

A_SCALE = float(np.float32(2.0 ** 7 / np.log(2.0)))   # 184.664
INV_A = float(np.float32(1.0 / A_SCALE))
MAGIC = 2.0 ** 7 * 127.0 - 5.58       # exp2 bit-trick constant (int16/bf16)

_CACHE = {}


def _build_nc(bias_val):
    import concourse.bacc as bacc
    import concourse.mybir as mybir
    from concourse.tile import TileContext

    f32 = mybir.dt.float32
    i16 = mybir.dt.int16
    bf16 = mybir.dt.bfloat16
    EXP = mybir.ActivationFunctionType.Exp
    ADD = mybir.AluOpType.add
    MAX = mybir.AluOpType.max
    bconst = float(np.float32(MAGIC - A_SCALE * bias_val))

    nc = bacc.Bacc("TRN2", target_bir_lowering=False, debug=False)

    qhh = nc.dram_tensor("qhh", [128, NQ], bf16, kind="ExternalInput")
    khs = nc.dram_tensor("khs", [128, NTOK], bf16, kind="ExternalInput")
    vpk = nc.dram_tensor("vpk", [128, NKT * 65], bf16, kind="ExternalInput")
    out = nc.dram_tensor("out", [65, NQ], f32, kind="ExternalOutput")

    GW = GKT * 128            # tokens per DMA group
    with TileContext(nc) as tc:
        with (
            tc.tile_pool(name="const", bufs=1) as const,
            tc.tile_pool(name="pa", bufs=3) as pa,
            tc.tile_pool(name="pd", bufs=3) as pd,
            tc.tile_pool(name="sbo", bufs=2) as sbo,
            tc.tile_pool(name="ps_s", bufs=6, space="PSUM") as ps_s,
            tc.tile_pool(name="ps_o", bufs=2, space="PSUM") as ps_o,
        ):
            # ---- PE + ACT warmup during the DMA prefix ----
            wz = const.tile([128, 512], bf16, tag="wz")
            nc.vector.memset(wz, 0.0)
            wexp = const.tile([128, 1], f32, tag="wexp")
            nc.scalar.activation(wexp, wz[:, 0:1], EXP)  # pull exp table load
            nbias_t = const.tile([128, 1], f32, tag="nbias")
            nc.vector.memset(nbias_t, -bias_val)
            for _ in range(10):
                wps = ps_s.tile([128, CHW], f32, tag="s")
                nc.tensor.matmul(wps[:, 0:256], wz[:, 0:128], wz[:, 0:256],
                                 start=True, stop=True)

            # ---- input DMAs (first superchunk's q, then k-side groups
            # interleaved with the rest) ----
            qhh_t = const.tile([128, NQ], bf16, tag="qhh")
            nc.sync.dma_start(out=qhh_t[:, 0:2 * CHW], in_=qhh[:, 0:2 * CHW])
            khs_g, vpk_g = [], []
            for g in range(NG):
                kt_ = const.tile([128, GW], bf16, tag=f"khs_{g}")
                nc.sync.dma_start(out=kt_, in_=khs[:, g * GW:(g + 1) * GW])
                khs_g.append(kt_)
                kt_ = const.tile([128, GKT * 65], bf16, tag=f"vpk_{g}")
                nc.sync.dma_start(
                    out=kt_, in_=vpk[:, g * GKT * 65:(g + 1) * GKT * 65])
                vpk_g.append(kt_)
                if g == 0:
                    nc.sync.dma_start(out=qhh_t[:, 2 * CHW:4 * CHW],
                                      in_=qhh[:, 2 * CHW:4 * CHW])

            # ---- main loop: 2 superchunks x (pair-major x 2 chunks) ----
            for sc in range(2):
                o_accs = [ps_o.tile([65, CHW], f32, tag="oacc",
                                    name=f"oacc{sc}{i}")
                          for i in range(2)]
                for pr in range(NPAIR):
                    ktA, ktB = 2 * pr, 2 * pr + 1
                    g = ktA // GKT
                    lA = (ktA - g * GKT) * 128
                    lB = (ktB - g * GKT) * 128
                    for ci in range(2):
                        ch = 2 * sc + ci
                        qs = slice(ch * CHW, (ch + 1) * CHW)
                        o_acc = o_accs[ci]
                        # z = (A*K) . q, bf16, packed pair (ktA on PE rows
                        # 0-63 / ktB on 64-127, streaming concurrently).
                        # One 1-bank s tile per key tile: exp of the two
                        # halves runs on ScalarE and VectorE in parallel,
                        # and the 6-deep ring keeps the PE queue fed.
                        s_a = ps_s.tile([128, CHW], f32, tag="s", name="s_a")
                        s_b = ps_s.tile([128, CHW], f32, tag="s", name="s_b")
                        nc.tensor.matmul(
                            s_a[:, :],
                            khs_g[g][0:64, lA:lA + 128], qhh_t[0:64, qs],
                            start=True, stop=True,
                        )
                        nc.tensor.matmul(
                            s_b[:, :],
                            khs_g[g][64:128, lB:lB + 128], qhh_t[64:128, qs],
                            start=True, stop=True,
                        )
                        # ScalarE: p = exp(z/A - bias), exact (tile A)
                        p_t = pa.tile([128, CHW], bf16, tag="p_act")
                        nc.scalar.activation(p_t, s_a, EXP,
                                             bias=nbias_t[:, 0:1],
                                             scale=INV_A)
                        # VectorE: Schraudolph bits = int(max(z + b, 0))
                        p_i = pd.tile([128, CHW], i16, tag="p_dve")
                        nc.vector.tensor_scalar(
                            p_i[:, :], s_b[:, :], bconst, 0.0, ADD, MAX)
                        for half, kt, p_use in (
                            (0, ktA, p_t[:, :]),
                            (1, ktB, p_i[:, :].bitcast(bf16)),
                        ):
                            lv = (kt - g * GKT) * 65
                            nc.tensor.matmul(
                                o_acc[:, :],
                                vpk_g[g][:, lv:lv + 65],
                                p_use,
                                start=(pr == 0 and half == 0),
                                stop=(pr == NPAIR - 1 and half == 1),
                                skip_group_check=True,
                            )
                # ---- ship O^T chunks (normalize + transpose on host) ----
                for ci in range(2):
                    ch = 2 * sc + ci
                    qs = slice(ch * CHW, (ch + 1) * CHW)
                    o_sb = sbo.tile([65, CHW], f32, tag="osb")
                    nc.vector.tensor_copy(o_sb, o_accs[ci])
                    nc.sync.dma_start(out=out[:, qs], in_=o_sb)
    nc.compile()
    return nc


def _prep_inputs(x):
    """Host-side shard + operand marshaling. Returns (in_maps, bias_val)."""
    import ml_dtypes
    bf16 = ml_dtypes.bfloat16
    t = np.ascontiguousarray(x, np.float32).reshape(B_, NTOK, C_)
    smax = float((t.astype(np.float64) ** 2).sum(-1).max())
    bias_val = float(np.float32(max(32.0, smax - 70.0)))
    in_maps = []
    for b in range(B_):
        kv = t[b]                                   # [4096, 64]
        ks = (kv.T * np.float32(A_SCALE)).astype(bf16)
        khs = np.concatenate([ks, ks])              # [128, 4096]
        vpk = np.concatenate(
            [np.concatenate([kv[i * 128:(i + 1) * 128],
                             np.ones((128, 1), np.float32)], axis=1)
             for i in range(NKT)], axis=1).astype(bf16)  # [128, 32*65]
        for h in range(2):
            q = t[b, h * NQ:(h + 1) * NQ]           # [2048, 64]
            qhh = np.concatenate([q.T, q.T]).astype(bf16)
            in_maps.append({"qhh": qhh, "khs": khs, "vpk": vpk})
    return in_maps, bias_val


def run(x, trace=False):
    from concourse.bass_utils import run_bass_kernel_spmd
    in_maps, bias_val = _prep_inputs(x)
    if _CACHE.get("bias") != bias_val:
        _CACHE["nc"] = _build_nc(bias_val)
        _CACHE["bias"] = bias_val
    nc = _CACHE["nc"]
    res = run_bass_kernel_spmd(
        nc, in_maps, core_ids=list(range(NCORES)), trace=trace,
    )
    full = np.empty((B_, NTOK, C_), np.float32)
    for b in range(B_):
        for h in range(2):
            o = res.results[2 * b + h]["out"]        # [65, 2048]
            full[b, h * NQ:(h + 1) * NQ] = (o[0:C_] / o[C_]).T
    return full.reshape(B_, D_, H_, W_, C_), res


def kernel(x):
    out, _ = run(x, trace=False)
    return out


# revision 11
# speedup vs baseline: 1.3503x; 1.3503x over previous
"""Trainium2 Bass kernel for nn_Denoiser_73598559584966.

Full-sequence self-attention (Q=K=V, no scaling) over x: [4, 16, 16, 16, 64]
  t = x.reshape(B, 4096, 64); out = softmax(t @ t^T) @ t

Sharding: 8 cores = 4 batches x 2 query-halves. Each core: 2048 queries
vs the full 4096 keys/values of its batch. No collectives.

Device algorithm per core (scores kept transposed: [keys, queries]),
all matmul operands bf16, PSUM accumulation fp32. K is pre-scaled on
host by A = 2^7/ln2 so the QK matmul directly produces z = A*s (s in
log2 units scaled to the bf16 exponent grid). Key tiles processed in
packed pairs (ktA on PE rows 0-63, ktB on rows 64-127: the two
contraction-64 matmuls run concurrently in disjoint row-halves):
  z[kt, q] = (A*K)_kt . q          bf16, contraction 64, one pass
  p = exp(s - bias)   half tiles:  ScalarE  exp(z*(1/A) - bias) -> bf16
                      other half:  VectorE  Schraudolph fast exp:
                                   i16 = int(max(z + b, 0)); p = bits(i16)
                                   read as bf16 (exponent+7-bit mantissa;
                                   ~3% rel err on p, harmless: softmax
                                   here is near-one-hot on the diagonal)
  O^T[65, q] += (V_kt|1)^T P_kt    bf16 x bf16 -> fp32; row 64 = denom
Device returns O^T [65, 2048]; the host epilogue divides rows 0..63 by
row 64 and transposes while gathering shards (O(N*C) marshaling).

bias is chosen on host from max token norm^2 (the dominant diagonal
score) so both exp paths stay in range: z + b in (0, 32640) for every
realizable score, and exp(s - bias) cannot overflow.

Loop order is key-pair-major over 2-chunk superchunks so the first
pairs only need the first khs/vpk DMA group; later groups stream in
under compute (input DMA ~2 MB/core).
"""
import numpy as np

B_, D_, H_, W_, C_ = 4, 16, 16, 16, 64
NTOK = D_ * H_ * W_          # 4096 tokens per batch
NQ = NTOK // 2               # 2048 queries per core
NCORES = 8
NKT = NTOK // 128            # 32 key tiles
NPAIR = NKT // 2             # 16 packed key-tile pairs
NCH = 4                      # query chunks per core
CHW = NQ // NCH              # 512 queries per chunk
NG = 4                       # DMA groups over key tiles
GKT = NKT // NG              # 8 key tiles per group

A_SCALE = float(np.float32(2.0 ** 7 / np.log(2.0)))   # 184.664
INV_A = float(np.float32(1.0 / A_SCALE))
MAGIC = 2.0 ** 7 * 127.0 - 5.58       # exp2 bit-trick constant (int16/bf16)

_CACHE = {}


def _build_nc(bias_val):
    import concourse.bacc as bacc
    import concourse.mybir as mybir
    from concourse.tile import TileContext

    f32 = mybir.dt.float32
    i16 = mybir.dt.int16
    bf16 = mybir.dt.bfloat16
    EXP = mybir.ActivationFunctionType.Exp
    ADD = mybir.AluOpType.add
    MAX = mybir.AluOpType.max
    bconst = float(np.float32(MAGIC - A_SCALE * bias_val))

    nc = bacc.Bacc("TRN2", target_bir_lowering=False, debug=False)

    qhh = nc.dram_tensor("qhh", [128, NQ], bf16, kind="ExternalInput")
    khs = nc.dram_tensor("khs", [128, NTOK], bf16, kind="ExternalInput")
    vpk = nc.dram_tensor("vpk", [128, NKT * 65], bf16, kind="ExternalInput")
    out = nc.dram_tensor("out", [65, NQ], f32, kind="ExternalOutput")

    GW = GKT * 128            # tokens per DMA group
    with TileContext(nc) as tc:
        with (
            tc.tile_pool(name="const", bufs=1) as const,
            tc.tile_pool(name="pa", bufs=3) as pa,
            tc.tile_pool(name="pd", bufs=3) as pd,
            tc.tile_pool(name="sbo", bufs=2) as sbo,
            tc.tile_pool(name="ps_s", bufs=3, space="PSUM") as ps_s,
            tc.tile_pool(name="ps_o", bufs=2, space="PSUM") as ps_o,
        ):
            # ---- PE + ACT warmup during the DMA prefix ----
            wz = const.tile([128, 512], bf16, tag="wz")
            nc.vector.memset(wz, 0.0)
            wexp = const.tile([128, 1], f32, tag="wexp")
            nc.scalar.activation(wexp, wz[:, 0:1], EXP)  # pull exp table load
            nbias_t = const.tile([128, 1], f32, tag="nbias")
            nc.vector.memset(nbias_t, -bias_val)
            for _ in range(10):
                wps = ps_s.tile([128, 2 * CHW], f32, tag="s")
                nc.tensor.matmul(wps[:, 0:256], wz[:, 0:128], wz[:, 0:256],
                                 start=True, stop=True)

            # ---- input DMAs (first superchunk's q, then k-side groups
            # interleaved with the rest) ----
            qhh_t = const.tile([128, NQ], bf16, tag="qhh")
            nc.sync.dma_start(out=qhh_t[:, 0:2 * CHW], in_=qhh[:, 0:2 * CHW])
            khs_g, vpk_g = [], []
            for g in range(NG):
                kt_ = const.tile([128, GW], bf16, tag=f"khs_{g}")
                nc.sync.dma_start(out=kt_, in_=khs[:, g * GW:(g + 1) * GW])
                khs_g.append(kt_)
                kt_ = const.tile([128, GKT * 65], bf16, tag=f"vpk_{g}")
                nc.sync.dma_start(
                    out=kt_, in_=vpk[:, g * GKT * 65:(g + 1) * GKT * 65])
                vpk_g.append(kt_)
                if g == 0:
                    nc.sync.dma_start(out=qhh_t[:, 2 * CHW:4 * CHW],
                                      in_=qhh[:, 2 * CHW:4 * CHW])

            # ---- main loop: 2 superchunks x (pair-major x 2 chunks) ----
            for sc in range(2):
                o_accs = [ps_o.tile([65, CHW], f32, tag="oacc",
                                    name=f"oacc{sc}{i}")
                          for i in range(2)]
                for pr in range(NPAIR):
                    ktA, ktB = 2 * pr, 2 * pr + 1
                    g = ktA // GKT
                    lA = (ktA - g * GKT) * 128
                    lB = (ktB - g * GKT) * 128
                    for ci in range(2):
                        ch = 2 * sc + ci
                        qs = slice(ch * CHW, (ch + 1) * CHW)
                        o_acc = o_accs[ci]
                        # z = (A*K) . q, bf16, packed pair (ktA on PE rows
                        # 0-63 / ktB on 64-127, streaming concurrently into
                        # the two banks of one s tile). exp of the halves
                        # runs on ScalarE and VectorE in parallel (PSUM
                        # allows both engines on different banks).
                        s_t = ps_s.tile([128, 2 * CHW], f32, tag="s")
                        nc.tensor.matmul(
                            s_t[:, 0:CHW],
                            khs_g[g][0:64, lA:lA + 128], qhh_t[0:64, qs],
                            start=True, stop=True,
                        )
                        nc.tensor.matmul(
                            s_t[:, CHW:2 * CHW],
                            khs_g[g][64:128, lB:lB + 128], qhh_t[64:128, qs],
                            start=True, stop=True,
                        )
                        # ScalarE: p = exp(z/A - bias), exact (tile A)
                        p_t = pa.tile([128, CHW], bf16, tag="p_act")
                        nc.scalar.activation(p_t, s_t[:, 0:CHW], EXP,
                                             bias=nbias_t[:, 0:1],
                                             scale=INV_A)
                        # VectorE: Schraudolph bits = int(max(z + b, 0))
                        p_i = pd.tile([128, CHW], i16, tag="p_dve")
                        nc.vector.tensor_scalar(
                            p_i[:, :], s_t[:, CHW:2 * CHW],
                            bconst, 0.0, ADD, MAX)
                        for half, kt, p_use in (
                            (0, ktA, p_t[:, :]),
                            (1, ktB, p_i[:, :].bitcast(bf16)),
                        ):
                            lv = (kt - g * GKT) * 65
                            nc.tensor.matmul(
                                o_acc[:, :],
                                vpk_g[g][:, lv:lv + 65],
                                p_use,
                                start=(pr == 0 and half == 0),
                                stop=(pr == NPAIR - 1 and half == 1),
                                skip_group_check=True,
                            )
                # ---- ship O^T chunks (normalize + transpose on host) ----
                for ci in range(2):
                    ch = 2 * sc + ci
                    qs = slice(ch * CHW, (ch + 1) * CHW)
                    o_sb = sbo.tile([65, CHW], f32, tag="osb")
                    nc.vector.tensor_copy(o_sb, o_accs[ci])
                    nc.sync.dma_start(out=out[:, qs], in_=o_sb)
    nc.compile()
    return nc


def _prep_inputs(x):
    """Host-side shard + operand marshaling. Returns (in_maps, bias_val)."""
    import ml_dtypes
    bf16 = ml_dtypes.bfloat16
    t = np.ascontiguousarray(x, np.float32).reshape(B_, NTOK, C_)
    smax = float((t.astype(np.float64) ** 2).sum(-1).max())
    bias_val = float(np.float32(max(32.0, smax - 70.0)))
    in_maps = []
    for b in range(B_):
        kv = t[b]                                   # [4096, 64]
        ks = (kv.T * np.float32(A_SCALE)).astype(bf16)
        khs = np.concatenate([ks, ks])              # [128, 4096]
        vpk = np.concatenate(
            [np.concatenate([kv[i * 128:(i + 1) * 128],
                             np.ones((128, 1), np.float32)], axis=1)
             for i in range(NKT)], axis=1).astype(bf16)  # [128, 32*65]
        for h in range(2):
            q = t[b, h * NQ:(h + 1) * NQ]           # [2048, 64]
            qhh = np.concatenate([q.T, q.T]).astype(bf16)
            in_maps.append({"qhh": qhh, "khs": khs, "vpk": vpk})
    return in_maps, bias_val


def run(x, trace=False):
    from concourse.bass_utils import run_bass_kernel_spmd
    in_maps, bias_val = _prep_inputs(x)
    if _CACHE.get("bias") != bias_val:
        _CACHE["nc"] = _build_nc(bias_val)
        _CACHE["bias"] = bias_val
    nc = _CACHE["nc"]
    res = run_bass_kernel_spmd(
        nc, in_maps, core_ids=list(range(NCORES)), trace=trace,
    )
    full = np.empty((B_, NTOK, C_), np.float32)
    for b in range(B_):
        for h in range(2):
            o = res.results[2 * b + h]["out"]        # [65, 2048]
            full[b, h * NQ:(h + 1) * NQ] = (o[0:C_] / o[C_]).T
    return full.reshape(B_, D_, H_, W_, C_), res


def kernel(x):
    out, _ = run(x, trace=False)
    return out


# revision 15
# speedup vs baseline: 1.4146x; 1.0476x over previous
"""Trainium2 Bass kernel for nn_Denoiser_73598559584966.

Full-sequence self-attention (Q=K=V, no scaling) over x: [4, 16, 16, 16, 64]
  t = x.reshape(B, 4096, 64); out = softmax(t @ t^T) @ t

Sharding: 8 cores = 4 batches x 2 query-halves. Each core: 2048 queries
vs the full 4096 keys/values of its batch. No collectives.

Device algorithm per core (scores kept transposed: [keys, queries]),
all matmul operands bf16, PSUM accumulation fp32. K is pre-scaled on
host by A = 2^7/ln2 so the QK matmul directly produces z = A*s (s in
log2 units scaled to the bf16 exponent grid). Key tiles processed in
packed pairs (ktA on PE rows 0-63, ktB on rows 64-127: the two
contraction-64 matmuls run concurrently in disjoint row-halves):
  z[kt, q] = (A*K)_kt . q          bf16, contraction 64, one pass
  p = exp(s - bias)   half tiles:  ScalarE  exp(z*(1/A) - bias) -> bf16
                      other half:  VectorE  Schraudolph fast exp:
                                   i16 = int(max(z + b, 0)); p = bits(i16)
                                   read as bf16 (exponent+7-bit mantissa;
                                   ~3% rel err on p, harmless: softmax
                                   here is near-one-hot on the diagonal)
  O^T[65, q] += (V_kt|1)^T P_kt    bf16 x bf16 -> fp32; row 64 = denom
Device returns O^T [65, 2048]; the host epilogue divides rows 0..63 by
row 64 and transposes while gathering shards (O(N*C) marshaling).

bias is chosen on host from max token norm^2 (the dominant diagonal
score) so both exp paths stay in range: z + b in (0, 32640) for every
realizable score, and exp(s - bias) cannot overflow.

Loop order is key-pair-major over 2-chunk superchunks so the first
pairs only need the first khs/vpk DMA group; later groups stream in
under compute (input DMA ~2 MB/core).
"""
import numpy as np

B_, D_, H_, W_, C_ = 4, 16, 16, 16, 64
NTOK = D_ * H_ * W_          # 4096 tokens per batch
NQ = NTOK // 2               # 2048 queries per core
NCORES = 8
NKT = NTOK // 128            # 32 key tiles
NPAIR = NKT // 2             # 16 packed key-tile pairs
NCH = 4                      # query chunks per core
CHW = NQ // NCH              # 512 queries per chunk
NG = 4                       # DMA groups over key tiles
GKT = NKT // NG              # 8 key tiles per group

A_SCALE = float(np.float32(2.0 ** 7 / np.log(2.0)))   # 184.664
INV_A = float(np.float32(1.0 / A_SCALE))
MAGIC = 2.0 ** 7 * 127.0 - 5.58       # exp2 bit-trick constant (int16/bf16)

_CACHE = {}


def _build_nc(bias_val):
    import concourse.bacc as bacc
    import concourse.mybir as mybir
    from concourse.tile import TileContext

    f32 = mybir.dt.float32
    i16 = mybir.dt.int16
    bf16 = mybir.dt.bfloat16
    EXP = mybir.ActivationFunctionType.Exp
    ADD = mybir.AluOpType.add
    MAX = mybir.AluOpType.max
    bconst = float(np.float32(MAGIC - A_SCALE * bias_val))

    nc = bacc.Bacc("TRN2", target_bir_lowering=False, debug=False)

    qhh = nc.dram_tensor("qhh", [128, NQ], bf16, kind="ExternalInput")
    khs = nc.dram_tensor("khs", [128, NTOK], bf16, kind="ExternalInput")
    vpk = nc.dram_tensor("vpk", [128, NKT * 65], bf16, kind="ExternalInput")
    out = nc.dram_tensor("out", [65, NQ], f32, kind="ExternalOutput")

    GW = GKT * 128            # tokens per DMA group
    with TileContext(nc) as tc:
        with (
            tc.tile_pool(name="const", bufs=1) as const,
            tc.tile_pool(name="pa", bufs=4) as pa,
            tc.tile_pool(name="pd", bufs=4) as pd,
            tc.tile_pool(name="sbo", bufs=2) as sbo,
            tc.tile_pool(name="ps_s", bufs=3, space="PSUM") as ps_s,
            tc.tile_pool(name="ps_o", bufs=2, space="PSUM") as ps_o,
        ):
            # ---- input DMAs first (first superchunk's q + key groups,
            # interleaved with the rest) so the Sync queue starts early ----
            qhh_t = const.tile([128, NQ], bf16, tag="qhh")
            nc.sync.dma_start(out=qhh_t[:, 0:2 * CHW], in_=qhh[:, 0:2 * CHW])
            khs_g, vpk_g = [], []
            for g in range(NG):
                kt_ = const.tile([128, GW], bf16, tag=f"khs_{g}")
                nc.sync.dma_start(out=kt_, in_=khs[:, g * GW:(g + 1) * GW])
                khs_g.append(kt_)
                kt_ = const.tile([128, GKT * 65], bf16, tag=f"vpk_{g}")
                nc.sync.dma_start(
                    out=kt_, in_=vpk[:, g * GKT * 65:(g + 1) * GKT * 65])
                vpk_g.append(kt_)
                if g == 0:
                    nc.sync.dma_start(out=qhh_t[:, 2 * CHW:4 * CHW],
                                      in_=qhh[:, 2 * CHW:4 * CHW])

            # ---- PE + ACT warmup during the DMA prefix ----
            wz = const.tile([128, 512], bf16, tag="wz")
            nc.vector.memset(wz, 0.0)
            wexp = const.tile([128, 1], f32, tag="wexp")
            nc.scalar.activation(wexp, wz[:, 0:1], EXP)  # pull exp table load
            nbias_t = const.tile([128, 1], f32, tag="nbias")
            nc.vector.memset(nbias_t, -bias_val)
            for _ in range(10):
                wps = ps_s.tile([128, 2 * CHW], f32, tag="s")
                nc.tensor.matmul(wps[:, 0:256], wz[:, 0:128], wz[:, 0:256],
                                 start=True, stop=True)

            # ---- main loop: 2 superchunks x (pair-major x 2 chunks).
            # PV matmuls are emitted two iterations behind their QK pair
            # (software pipelining): by PV emission time its p operand is
            # long done, so the PE queue never stalls on exp and the
            # scheduler keeps QK pairs adjacent (concurrent streams). ----
            def emit_pv(it):
                o_acc, g, ktA, ktB, p_t, p_i, pr = it
                for half, kt, p_use in (
                    (0, ktA, p_t[:, :]),
                    (1, ktB, p_i[:, :].bitcast(bf16)),
                ):
                    lv = (kt - g * GKT) * 65
                    nc.tensor.matmul(
                        o_acc[:, :],
                        vpk_g[g][:, lv:lv + 65],
                        p_use,
                        start=(pr == 0 and half == 0),
                        stop=(pr == NPAIR - 1 and half == 1),
                        skip_group_check=True,
                    )

            for sc in range(2):
                o_accs = [ps_o.tile([65, CHW], f32, tag="oacc",
                                    name=f"oacc{sc}{i}")
                          for i in range(2)]
                pending = []
                for pr in range(NPAIR):
                    ktA, ktB = 2 * pr, 2 * pr + 1
                    g = ktA // GKT
                    lA = (ktA - g * GKT) * 128
                    lB = (ktB - g * GKT) * 128
                    for ci in range(2):
                        ch = 2 * sc + ci
                        qs = slice(ch * CHW, (ch + 1) * CHW)
                        # z = (A*K) . q, bf16, packed pair (ktA on PE rows
                        # 0-63 / ktB on 64-127, streaming concurrently into
                        # the two banks of one s tile). exp of the halves
                        # runs on ScalarE and VectorE in parallel (PSUM
                        # allows both engines on different banks).
                        s_t = ps_s.tile([128, 2 * CHW], f32, tag="s")
                        nc.tensor.matmul(
                            s_t[:, 0:CHW],
                            khs_g[g][0:64, lA:lA + 128], qhh_t[0:64, qs],
                            start=True, stop=True,
                        )
                        nc.tensor.matmul(
                            s_t[:, CHW:2 * CHW],
                            khs_g[g][64:128, lB:lB + 128], qhh_t[64:128, qs],
                            start=True, stop=True,
                        )
                        # ScalarE: p = exp(z/A - bias), exact (tile A)
                        p_t = pa.tile([128, CHW], bf16, tag="p_act")
                        nc.scalar.activation(p_t, s_t[:, 0:CHW], EXP,
                                             bias=nbias_t[:, 0:1],
                                             scale=INV_A)
                        # VectorE: Schraudolph bits = int(max(z + b, 0))
                        p_i = pd.tile([128, CHW], i16, tag="p_dve")
                        nc.vector.tensor_scalar(
                            p_i[:, :], s_t[:, CHW:2 * CHW],
                            bconst, 0.0, ADD, MAX)
                        pending.append(
                            (o_accs[ci], g, ktA, ktB, p_t, p_i, pr))
                        if len(pending) > 2:
                            emit_pv(pending.pop(0))
                for it in pending:
                    emit_pv(it)
                # ---- ship O^T chunks (normalize + transpose on host) ----
                for ci in range(2):
                    ch = 2 * sc + ci
                    qs = slice(ch * CHW, (ch + 1) * CHW)
                    o_sb = sbo.tile([65, CHW], f32, tag="osb")
                    nc.vector.tensor_copy(o_sb, o_accs[ci])
                    nc.sync.dma_start(out=out[:, qs], in_=o_sb)
    nc.compile()
    return nc


def _prep_inputs(x):
    """Host-side shard + operand marshaling. Returns (in_maps, bias_val)."""
    import ml_dtypes
    bf16 = ml_dtypes.bfloat16
    t = np.ascontiguousarray(x, np.float32).reshape(B_, NTOK, C_)
    smax = float((t.astype(np.float64) ** 2).sum(-1).max())
    bias_val = float(np.float32(max(32.0, smax - 70.0)))
    in_maps = []
    for b in range(B_):
        kv = t[b]                                   # [4096, 64]
        ks = (kv.T * np.float32(A_SCALE)).astype(bf16)
        khs = np.concatenate([ks, ks])              # [128, 4096]
        vpk = np.concatenate(
            [np.concatenate([kv[i * 128:(i + 1) * 128],
                             np.ones((128, 1), np.float32)], axis=1)
             for i in range(NKT)], axis=1).astype(bf16)  # [128, 32*65]
        for h in range(2):
            q = t[b, h * NQ:(h + 1) * NQ]           # [2048, 64]
            qhh = np.concatenate([q.T, q.T]).astype(bf16)
            in_maps.append({"qhh": qhh, "khs": khs, "vpk": vpk})
    return in_maps, bias_val


def run(x, trace=False):
    from concourse.bass_utils import run_bass_kernel_spmd
    in_maps, bias_val = _prep_inputs(x)
    if _CACHE.get("bias") != bias_val:
        _CACHE["nc"] = _build_nc(bias_val)
        _CACHE["bias"] = bias_val
    nc = _CACHE["nc"]
    res = run_bass_kernel_spmd(
        nc, in_maps, core_ids=list(range(NCORES)), trace=trace,
    )
    full = np.empty((B_, NTOK, C_), np.float32)
    for b in range(B_):
        for h in range(2):
            o = res.results[2 * b + h]["out"]        # [65, 2048]
            full[b, h * NQ:(h + 1) * NQ] = (o[0:C_] / o[C_]).T
    return full.reshape(B_, D_, H_, W_, C_), res


def kernel(x):
    out, _ = run(x, trace=False)
    return out


# revision 17
# speedup vs baseline: 1.4187x; 1.0029x over previous
"""Trainium2 Bass kernel for nn_Denoiser_73598559584966.

Full-sequence self-attention (Q=K=V, no scaling) over x: [4, 16, 16, 16, 64]
  t = x.reshape(B, 4096, 64); out = softmax(t @ t^T) @ t

Sharding: 8 cores = 4 batches x 2 query-halves. Each core: 2048 queries
vs the full 4096 keys/values of its batch. No collectives.

Device algorithm per core (scores kept transposed: [keys, queries]),
all matmul operands bf16, PSUM accumulation fp32. K is pre-scaled on
host by A = 2^7/ln2 so the QK matmul directly produces z = A*s (s in
log2 units scaled to the bf16 exponent grid). Key tiles processed in
packed pairs (ktA on PE rows 0-63, ktB on rows 64-127: the two
contraction-64 matmuls run concurrently in disjoint row-halves):
  z[kt, q] = (A*K)_kt . q          bf16, contraction 64, one pass
  p = exp(s - bias)   half tiles:  ScalarE  exp(z*(1/A) - bias) -> bf16
                      other half:  VectorE  Schraudolph fast exp:
                                   i16 = int(max(z + b, 0)); p = bits(i16)
                                   read as bf16 (exponent+7-bit mantissa;
                                   ~3% rel err on p, harmless: softmax
                                   here is near-one-hot on the diagonal)
  O^T[65, q] += (V_kt|1)^T P_kt    bf16 x bf16 -> fp32; row 64 = denom
Device returns O^T [65, 2048]; the host epilogue divides rows 0..63 by
row 64 and transposes while gathering shards (O(N*C) marshaling).

bias is chosen on host from max token norm^2 (the dominant diagonal
score) so both exp paths stay in range: z + b in (0, 32640) for every
realizable score, and exp(s - bias) cannot overflow.

Loop order is key-pair-major over 2-chunk superchunks so the first
pairs only need the first khs/vpk DMA group; later groups stream in
under compute (input DMA ~2 MB/core).
"""
import numpy as np

B_, D_, H_, W_, C_ = 4, 16, 16, 16, 64
NTOK = D_ * H_ * W_          # 4096 tokens per batch
NQ = NTOK // 2               # 2048 queries per core
NCORES = 8
NKT = NTOK // 128            # 32 key tiles
NPAIR = NKT // 2             # 16 packed key-tile pairs
NCH = 4                      # query chunks per core
CHW = NQ // NCH              # 512 queries per chunk
NG = 4                       # DMA groups over key tiles
GKT = NKT // NG              # 8 key tiles per group

A_SCALE = float(np.float32(2.0 ** 7 / np.log(2.0)))   # 184.664
INV_A = float(np.float32(1.0 / A_SCALE))
MAGIC = 2.0 ** 7 * 127.0 - 5.58       # exp2 bit-trick constant (int16/bf16)

_CACHE = {}


def _build_nc(bias_val):
    import concourse.bacc as bacc
    import concourse.mybir as mybir
    from concourse.tile import TileContext

    f32 = mybir.dt.float32
    i16 = mybir.dt.int16
    bf16 = mybir.dt.bfloat16
    EXP = mybir.ActivationFunctionType.Exp
    ADD = mybir.AluOpType.add
    MAX = mybir.AluOpType.max
    bconst = float(np.float32(MAGIC - A_SCALE * bias_val))

    nc = bacc.Bacc("TRN2", target_bir_lowering=False, debug=False)

    qhh = nc.dram_tensor("qhh", [128, NQ], bf16, kind="ExternalInput")
    khs = nc.dram_tensor("khs", [128, NTOK], bf16, kind="ExternalInput")
    vpk = nc.dram_tensor("vpk", [128, NKT * 65], bf16, kind="ExternalInput")
    out = nc.dram_tensor("out", [65, NQ], f32, kind="ExternalOutput")

    GW = GKT * 128            # tokens per DMA group
    with TileContext(nc) as tc:
        with (
            tc.tile_pool(name="const", bufs=1) as const,
            tc.tile_pool(name="pa", bufs=5) as pa,
            tc.tile_pool(name="pd", bufs=5) as pd,
            tc.tile_pool(name="sbo", bufs=2) as sbo,
            tc.tile_pool(name="ps_s", bufs=3, space="PSUM") as ps_s,
            tc.tile_pool(name="ps_o", bufs=2, space="PSUM") as ps_o,
        ):
            # ---- input DMAs first (needed-first order: khs0, q01, vpk0;
            # remaining groups stream in under compute) ----
            qhh_t = const.tile([128, NQ], bf16, tag="qhh")
            khs_g, vpk_g = [], []
            for g in range(NG):
                kt_ = const.tile([128, GW], bf16, tag=f"khs_{g}",
                                 name=f"khs_t{g}")
                khs_g.append(kt_)
                kt_ = const.tile([128, GKT * 65], bf16, tag=f"vpk_{g}",
                                 name=f"vpk_t{g}")
                vpk_g.append(kt_)
            nc.sync.dma_start(out=khs_g[0], in_=khs[:, 0:GW])
            nc.sync.dma_start(out=qhh_t[:, 0:2 * CHW], in_=qhh[:, 0:2 * CHW])
            nc.sync.dma_start(out=vpk_g[0], in_=vpk[:, 0:GKT * 65])
            for g in range(1, NG):
                nc.sync.dma_start(out=khs_g[g], in_=khs[:, g * GW:(g + 1) * GW])
                nc.sync.dma_start(
                    out=vpk_g[g], in_=vpk[:, g * GKT * 65:(g + 1) * GKT * 65])
                if g == 1:
                    nc.sync.dma_start(out=qhh_t[:, 2 * CHW:4 * CHW],
                                      in_=qhh[:, 2 * CHW:4 * CHW])

            # ---- ACT table pull + bias constant during the DMA prefix
            # (no PE warmup matmuls: the first main pairs self-warm) ----
            wz = const.tile([128, 4], bf16, tag="wz")
            nc.vector.memset(wz, 0.0)
            wexp = const.tile([128, 1], f32, tag="wexp")
            nc.scalar.activation(wexp, wz[:, 0:1], EXP)  # pull exp table load
            nbias_t = const.tile([128, 1], f32, tag="nbias")
            nc.vector.memset(nbias_t, -bias_val)

            # ---- main loop: 2 superchunks x (pair-major x 2 chunks).
            # PV matmuls are emitted two iterations behind their QK pair
            # (software pipelining): by PV emission time its p operand is
            # long done, so the PE queue never stalls on exp and the
            # scheduler keeps QK pairs adjacent (concurrent streams). ----
            def emit_pv(it):
                o_acc, g, ktA, ktB, p_t, p_i, pr = it
                for half, kt, p_use in (
                    (0, ktA, p_t[:, :]),
                    (1, ktB, p_i[:, :].bitcast(bf16)),
                ):
                    lv = (kt - g * GKT) * 65
                    nc.tensor.matmul(
                        o_acc[:, :],
                        vpk_g[g][:, lv:lv + 65],
                        p_use,
                        start=(pr == 0 and half == 0),
                        stop=(pr == NPAIR - 1 and half == 1),
                        skip_group_check=True,
                    )

            for sc in range(2):
                o_accs = [ps_o.tile([65, CHW], f32, tag="oacc",
                                    name=f"oacc{sc}{i}")
                          for i in range(2)]
                pending = []
                for pr in range(NPAIR):
                    ktA, ktB = 2 * pr, 2 * pr + 1
                    g = ktA // GKT
                    lA = (ktA - g * GKT) * 128
                    lB = (ktB - g * GKT) * 128
                    for ci in range(2):
                        ch = 2 * sc + ci
                        qs = slice(ch * CHW, (ch + 1) * CHW)
                        # z = (A*K) . q, bf16, packed pair (ktA on PE rows
                        # 0-63 / ktB on 64-127, streaming concurrently into
                        # the two banks of one s tile). exp of the halves
                        # runs on ScalarE and VectorE in parallel (PSUM
                        # allows both engines on different banks).
                        s_t = ps_s.tile([128, 2 * CHW], f32, tag="s")
                        nc.tensor.matmul(
                            s_t[:, 0:CHW],
                            khs_g[g][0:64, lA:lA + 128], qhh_t[0:64, qs],
                            start=True, stop=True,
                        )
                        nc.tensor.matmul(
                            s_t[:, CHW:2 * CHW],
                            khs_g[g][64:128, lB:lB + 128], qhh_t[64:128, qs],
                            start=True, stop=True,
                        )
                        # ScalarE: p = exp(z/A - bias), exact (tile A)
                        p_t = pa.tile([128, CHW], bf16, tag="p_act")
                        nc.scalar.activation(p_t, s_t[:, 0:CHW], EXP,
                                             bias=nbias_t[:, 0:1],
                                             scale=INV_A)
                        # VectorE: Schraudolph bits = int(max(z + b, 0))
                        p_i = pd.tile([128, CHW], i16, tag="p_dve")
                        nc.vector.tensor_scalar(
                            p_i[:, :], s_t[:, CHW:2 * CHW],
                            bconst, 0.0, ADD, MAX)
                        pending.append(
                            (o_accs[ci], g, ktA, ktB, p_t, p_i, pr))
                        if len(pending) > 3:
                            emit_pv(pending.pop(0))
                for it in pending:
                    emit_pv(it)
                # ---- ship O^T chunks (normalize + transpose on host) ----
                for ci in range(2):
                    ch = 2 * sc + ci
                    qs = slice(ch * CHW, (ch + 1) * CHW)
                    o_sb = sbo.tile([65, CHW], f32, tag="osb")
                    nc.vector.tensor_copy(o_sb, o_accs[ci])
                    nc.sync.dma_start(out=out[:, qs], in_=o_sb)
    nc.compile()
    return nc


def _prep_inputs(x):
    """Host-side shard + operand marshaling. Returns (in_maps, bias_val)."""
    import ml_dtypes
    bf16 = ml_dtypes.bfloat16
    t = np.ascontiguousarray(x, np.float32).reshape(B_, NTOK, C_)
    smax = float((t.astype(np.float64) ** 2).sum(-1).max())
    bias_val = float(np.float32(max(32.0, smax - 70.0)))
    in_maps = []
    for b in range(B_):
        kv = t[b]                                   # [4096, 64]
        ks = (kv.T * np.float32(A_SCALE)).astype(bf16)
        khs = np.concatenate([ks, ks])              # [128, 4096]
        vpk = np.concatenate(
            [np.concatenate([kv[i * 128:(i + 1) * 128],
                             np.ones((128, 1), np.float32)], axis=1)
             for i in range(NKT)], axis=1).astype(bf16)  # [128, 32*65]
        for h in range(2):
            q = t[b, h * NQ:(h + 1) * NQ]           # [2048, 64]
            qhh = np.concatenate([q.T, q.T]).astype(bf16)
            in_maps.append({"qhh": qhh, "khs": khs, "vpk": vpk})
    return in_maps, bias_val


def run(x, trace=False):
    from concourse.bass_utils import run_bass_kernel_spmd
    in_maps, bias_val = _prep_inputs(x)
    if _CACHE.get("bias") != bias_val:
        _CACHE["nc"] = _build_nc(bias_val)
        _CACHE["bias"] = bias_val
    nc = _CACHE["nc"]
    res = run_bass_kernel_spmd(
        nc, in_maps, core_ids=list(range(NCORES)), trace=trace,
    )
    full = np.empty((B_, NTOK, C_), np.float32)
    for b in range(B_):
        for h in range(2):
            o = res.results[2 * b + h]["out"]        # [65, 2048]
            full[b, h * NQ:(h + 1) * NQ] = (o[0:C_] / o[C_]).T
    return full.reshape(B_, D_, H_, W_, C_), res


def kernel(x):
    out, _ = run(x, trace=False)
    return out


# revision 19
# speedup vs baseline: 1.5097x; 1.0642x over previous
"""Trainium2 Bass kernel for nn_Denoiser_73598559584966.

Full-sequence self-attention (Q=K=V, no scaling) over x: [4, 16, 16, 16, 64]
  t = x.reshape(B, 4096, 64); out = softmax(t @ t^T) @ t

Sharding: 8 cores = 4 batches x 2 query-halves. Each core: 2048 queries
vs the full 4096 keys/values of its batch. No collectives.

Device algorithm per core (scores kept transposed: [keys, queries]),
all matmul operands bf16, PSUM accumulation fp32. K is pre-scaled on
host by A = 2^7/ln2 so the QK matmul directly produces z = A*s (s in
log2 units scaled to the bf16 exponent grid). Key tiles processed in
packed pairs (ktA on PE rows 0-63, ktB on rows 64-127: the two
contraction-64 matmuls run concurrently in disjoint row-halves):
  z[kt, q] = (A*K)_kt . q          bf16, contraction 64, one pass
  p = exp(s - bias)   half tiles:  ScalarE  exp(z*(1/A) - bias) -> bf16
                      other half:  VectorE  Schraudolph fast exp:
                                   i16 = int(max(z + b, 0)); p = bits(i16)
                                   read as bf16 (exponent+7-bit mantissa;
                                   ~3% rel err on p, harmless: softmax
                                   here is near-one-hot on the diagonal)
  O^T[65, q] += (V_kt|1)^T P_kt    bf16 x bf16 -> fp32; row 64 = denom
Device returns O^T [65, 2048]; the host epilogue divides rows 0..63 by
row 64 and transposes while gathering shards (O(N*C) marshaling).

bias is chosen on host from max token norm^2 (the dominant diagonal
score) so both exp paths stay in range: z + b in (0, 32640) for every
realizable score, and exp(s - bias) cannot overflow.

Loop order is key-pair-major over 2-chunk superchunks so the first
pairs only need the first khs/vpk DMA group; later groups stream in
under compute (input DMA ~2 MB/core).
"""
import numpy as np

B_, D_, H_, W_, C_ = 4, 16, 16, 16, 64
NTOK = D_ * H_ * W_          # 4096 tokens per batch
NQ = NTOK // 2               # 2048 queries per core
NCORES = 8
NKT = NTOK // 128            # 32 key tiles
NPAIR = NKT // 2             # 16 packed key-tile pairs
NCH = 4                      # query chunks per core
CHW = NQ // NCH              # 512 queries per chunk
NG = 4                       # DMA groups over key tiles
GKT = NKT // NG              # 8 key tiles per group

A_SCALE = float(np.float32(2.0 ** 7 / np.log(2.0)))   # 184.664
INV_A = float(np.float32(1.0 / A_SCALE))
MAGIC = 2.0 ** 7 * 127.0 - 5.58       # exp2 bit-trick constant (int16/bf16)

_CACHE = {}


def _build_nc(bias_val):
    import concourse.bacc as bacc
    import concourse.mybir as mybir
    from concourse.tile import TileContext

    f32 = mybir.dt.float32
    i16 = mybir.dt.int16
    bf16 = mybir.dt.bfloat16
    EXP = mybir.ActivationFunctionType.Exp
    ADD = mybir.AluOpType.add
    MAX = mybir.AluOpType.max
    bconst = float(np.float32(MAGIC - A_SCALE * bias_val))

    nc = bacc.Bacc("TRN2", target_bir_lowering=False, debug=False)

    qhh = nc.dram_tensor("qhh", [128, NQ], bf16, kind="ExternalInput")
    khs = nc.dram_tensor("khs", [128, NTOK], bf16, kind="ExternalInput")
    vpk = nc.dram_tensor("vpk", [128, NKT * 65], bf16, kind="ExternalInput")
    out = nc.dram_tensor("out", [65, NQ], f32, kind="ExternalOutput")

    GW = GKT * 128            # tokens per DMA group
    with TileContext(nc) as tc:
        with (
            tc.tile_pool(name="const", bufs=1) as const,
            tc.tile_pool(name="pa", bufs=6) as pa,
            tc.tile_pool(name="pd", bufs=6) as pd,
            tc.tile_pool(name="sbo", bufs=2) as sbo,
            tc.tile_pool(name="ps_s", bufs=3, space="PSUM") as ps_s,
            tc.tile_pool(name="ps_o", bufs=2, space="PSUM") as ps_o,
        ):
            # ---- input DMAs first (needed-first order: khs0, q01, vpk0;
            # remaining groups stream in under compute) ----
            qhh_t = const.tile([128, NQ], bf16, tag="qhh")
            khs_g, vpk_g = [], []
            for g in range(NG):
                kt_ = const.tile([128, GW], bf16, tag=f"khs_{g}",
                                 name=f"khs_t{g}")
                khs_g.append(kt_)
                kt_ = const.tile([128, GKT * 65], bf16, tag=f"vpk_{g}",
                                 name=f"vpk_t{g}")
                vpk_g.append(kt_)
            # tiny dedicated first transfers: exactly what pair 0 chunk 0
            # needs, so the first QK fires as early as possible
            nc.sync.dma_start(out=khs_g[0][:, 0:256], in_=khs[:, 0:256])
            nc.sync.dma_start(out=qhh_t[:, 0:CHW], in_=qhh[:, 0:CHW])
            nc.sync.dma_start(out=khs_g[0][:, 256:GW], in_=khs[:, 256:GW])
            nc.sync.dma_start(out=qhh_t[:, CHW:2 * CHW],
                              in_=qhh[:, CHW:2 * CHW])
            nc.sync.dma_start(out=vpk_g[0], in_=vpk[:, 0:GKT * 65])
            for g in range(1, NG):
                nc.sync.dma_start(out=khs_g[g], in_=khs[:, g * GW:(g + 1) * GW])
                nc.sync.dma_start(
                    out=vpk_g[g], in_=vpk[:, g * GKT * 65:(g + 1) * GKT * 65])
                if g == 1:
                    nc.sync.dma_start(out=qhh_t[:, 2 * CHW:4 * CHW],
                                      in_=qhh[:, 2 * CHW:4 * CHW])

            # ---- ACT table pull + bias constant during the DMA prefix
            # (no PE warmup matmuls: the first main pairs self-warm) ----
            wz = const.tile([128, 4], bf16, tag="wz")
            nc.vector.memset(wz, 0.0)
            wexp = const.tile([128, 1], f32, tag="wexp")
            nc.scalar.activation(wexp, wz[:, 0:1], EXP)  # pull exp table load
            nbias_t = const.tile([128, 1], f32, tag="nbias")
            nc.vector.memset(nbias_t, -bias_val)

            # ---- main loop: 2 superchunks x (pair-major x 2 chunks).
            # PV matmuls are emitted two iterations behind their QK pair
            # (software pipelining): by PV emission time its p operand is
            # long done, so the PE queue never stalls on exp and the
            # scheduler keeps QK pairs adjacent (concurrent streams). ----
            def emit_pv(it):
                o_acc, g, ktA, ktB, p_t, p_i, pr = it
                for half, kt, p_use in (
                    (0, ktA, p_t[:, :]),
                    (1, ktB, p_i[:, :].bitcast(bf16)),
                ):
                    lv = (kt - g * GKT) * 65
                    nc.tensor.matmul(
                        o_acc[:, :],
                        vpk_g[g][:, lv:lv + 65],
                        p_use,
                        start=(pr == 0 and half == 0),
                        stop=(pr == NPAIR - 1 and half == 1),
                        skip_group_check=True,
                    )

            for sc in range(2):
                o_accs = [ps_o.tile([65, CHW], f32, tag="oacc",
                                    name=f"oacc{sc}{i}")
                          for i in range(2)]
                pending = []
                for pr in range(NPAIR):
                    ktA, ktB = 2 * pr, 2 * pr + 1
                    g = ktA // GKT
                    lA = (ktA - g * GKT) * 128
                    lB = (ktB - g * GKT) * 128
                    for ci in range(2):  # both chunks' QK pairs back-to-back
                        ch = 2 * sc + ci
                        qs = slice(ch * CHW, (ch + 1) * CHW)
                        # z = (A*K) . q, bf16, packed pair (ktA on PE rows
                        # 0-63 / ktB on 64-127, streaming concurrently into
                        # the two banks of one s tile). exp of the halves
                        # runs on ScalarE and VectorE in parallel (PSUM
                        # allows both engines on different banks).
                        s_t = ps_s.tile([128, 2 * CHW], f32, tag="s")
                        nc.tensor.matmul(
                            s_t[:, 0:CHW],
                            khs_g[g][0:64, lA:lA + 128], qhh_t[0:64, qs],
                            start=True, stop=True,
                        )
                        nc.tensor.matmul(
                            s_t[:, CHW:2 * CHW],
                            khs_g[g][64:128, lB:lB + 128], qhh_t[64:128, qs],
                            start=True, stop=True,
                        )
                        # ScalarE: p = exp(z/A - bias), exact (tile A)
                        p_t = pa.tile([128, CHW], bf16, tag="p_act")
                        nc.scalar.activation(p_t, s_t[:, 0:CHW], EXP,
                                             bias=nbias_t[:, 0:1],
                                             scale=INV_A)
                        # VectorE: Schraudolph bits = int(max(z + b, 0))
                        p_i = pd.tile([128, CHW], i16, tag="p_dve")
                        nc.vector.tensor_scalar(
                            p_i[:, :], s_t[:, CHW:2 * CHW],
                            bconst, 0.0, ADD, MAX)
                        pending.append(
                            (o_accs[ci], g, ktA, ktB, p_t, p_i, pr))
                    while len(pending) > 2:
                        emit_pv(pending.pop(0))
                for it in pending:
                    emit_pv(it)
                # ---- ship O^T chunks (normalize + transpose on host) ----
                for ci in range(2):
                    ch = 2 * sc + ci
                    qs = slice(ch * CHW, (ch + 1) * CHW)
                    o_sb = sbo.tile([65, CHW], f32, tag="osb")
                    nc.vector.tensor_copy(o_sb, o_accs[ci])
                    nc.sync.dma_start(out=out[:, qs], in_=o_sb)
    nc.compile()
    return nc


def _prep_inputs(x):
    """Host-side shard + operand marshaling. Returns (in_maps, bias_val)."""
    import ml_dtypes
    bf16 = ml_dtypes.bfloat16
    t = np.ascontiguousarray(x, np.float32).reshape(B_, NTOK, C_)
    smax = float((t.astype(np.float64) ** 2).sum(-1).max())
    bias_val = float(np.float32(max(32.0, smax - 70.0)))
    in_maps = []
    for b in range(B_):
        kv = t[b]                                   # [4096, 64]
        ks = (kv.T * np.float32(A_SCALE)).astype(bf16)
        khs = np.concatenate([ks, ks])              # [128, 4096]
        vpk = np.concatenate(
            [np.concatenate([kv[i * 128:(i + 1) * 128],
                             np.ones((128, 1), np.float32)], axis=1)
             for i in range(NKT)], axis=1).astype(bf16)  # [128, 32*65]
        for h in range(2):
            q = t[b, h * NQ:(h + 1) * NQ]           # [2048, 64]
            qhh = np.concatenate([q.T, q.T]).astype(bf16)
            in_maps.append({"qhh": qhh, "khs": khs, "vpk": vpk})
    return in_maps, bias_val


def run(x, trace=False):
    from concourse.bass_utils import run_bass_kernel_spmd
    in_maps, bias_val = _prep_inputs(x)
    if _CACHE.get("bias") != bias_val:
        _CACHE["nc"] = _build_nc(bias_val)
        _CACHE["bias"] = bias_val
    nc = _CACHE["nc"]
    res = run_bass_kernel_spmd(
        nc, in_maps, core_ids=list(range(NCORES)), trace=trace,
    )
    full = np.empty((B_, NTOK, C_), np.float32)
    for b in range(B_):
        for h in range(2):
            o = res.results[2 * b + h]["out"]        # [65, 2048]
            full[b, h * NQ:(h + 1) * NQ] = (o[0:C_] / o[C_]).T
    return full.reshape(B_, D_, H_, W_, C_), res


def kernel(x):
    out, _ = run(x, trace=False)
    return out
